# revision 9
# baseline (speedup 1.0000x reference)
"""Gemma2 sliding-window attention (B=1, L=4096, H=8/KV4, D=256, HID=2304, W=2048)
on 8 TRN2 NeuronCores via Bass/Tile.

Key structural facts of the reference (validated against it numerically):
- The window mask keeps only key columns >= 2048 for ALL rows; combined with
  the causal mask, rows < 2048 end up with every logit == -1e9 exactly in fp32
  (|softcapped score| < 32 < ulp(1e9)/2), so softmax is uniform over all 4096
  keys: rows 0..2047 of the output are one constant row = colmean(v) @ wo.
- Rows >= 2048 are standard causal softcapped attention over keys [2048, i];
  the -1e9 terms underflow to exactly 0 in the fp32 softmax.
- Softcap bounds logits to [-50, 50], so exp() without max-subtraction is safe
  in fp32 and matches the reference softmax up to rounding.

Sharding: one query head per core. The K/V projections for kv head g=h//2 are
deduplicated across the pair (2g, 2g+1): the even core projects K^T, the odd
core projects V^T (identical SPMD programs; only the per-core weight data
differ), exchanged per 512-block with on-device pair AllGathers (a tiny warmup
collective at kernel start absorbs the ~30us one-time rendezvous). Both cores
then rope the gathered K^T (slot 0) with the shared rope tables and transpose
V^T (slot 1) into V [j, d] layout with PE transposes. kv/q projection blocks
are interleaved so the x2t input stream paces evenly, and warmup matmuls on
scratch data bring the PE to full clock before real work.

Phase 2 computes scores in [j_part, i_free] layout, unnormalized oT in PSUM,
projects through this head's wo slice (the previous block's 20 projection
chunks are spread 3-per-score-chunk through the loop to keep every queue
shallow), and normalizes per query row with a per-partition 1/denominator
scale fused into the PSUM->SBUF drain, alternating vector/scalar. The
denominator accumulates on gpsimd and collapses to a transposed [128, 4]
vector with four K=128 ones-matmuls per i-block. The gathered K/V blocks
unpack lazily inside phase 2 (block b is only needed near the end of phase-2
block b), hiding the collective latency. fp16 partials [2048, 2304] stream
out; host sums the 8 partials in fp32 and prepends the constant first-half
row.
"""
import sys

sys.path.insert(0, "/opt/trn_rl_repo")

import numpy as np
import ml_dtypes

H = 8
HKV = 4
D = 256
HID = 2304
L = 4096
LI = 2048          # second-half rows (local)
NCC = HID // 128   # 18 contraction chunks
NIB = LI // 512    # 4 i-blocks of 512
SCALE = (HID // H) ** -0.5
SOFTCAP = 50.0
NEG = -1e9
ROPE_BASE = 10000.0

_BF16 = ml_dtypes.bfloat16

_CACHE = {}

PAIRS = [[0, 1], [2, 3], [4, 5], [6, 7]]


def _hid_chunks():
    out = []
    c = 0
    while c < HID:
        w = min(512, HID - c)
        out.append((c, w))
        c += w
    return out


def _build_nc():
    import concourse.bass as bass
    import concourse.mybir as mybir
    import concourse.tile as tile
    from concourse import bacc

    f32 = mybir.dt.float32
    f16 = mybir.dt.float16
    bf16 = mybir.dt.bfloat16

    nc = bacc.Bacc("TRN2", target_bir_lowering=False, debug=False, num_devices=8)

    x2t_d = nc.dram_tensor("x2t", [HID, LI], f16, kind="ExternalInput").ap()
    wq_d = nc.dram_tensor("wq", [HID, D], f16, kind="ExternalInput").ap()
    wkv_d = nc.dram_tensor("wkv", [HID, D], f16, kind="ExternalInput").ap()
    wo_d = nc.dram_tensor("wo", [D, HID], f16, kind="ExternalInput").ap()
    # rope tables: rows d and d+128 of the [D, LI] table are identical, so
    # only the first 128 rows are stored and shared by both halves
    cos_d = nc.dram_tensor("cost", [128, LI], f16, kind="ExternalInput").ap()
    sin_d = nc.dram_tensor("sint", [128, LI], f16, kind="ExternalInput").ap()
    tri_d = nc.dram_tensor("tri", [128, 2048], bf16, kind="ExternalInput").ap()
    onesb_d = nc.dram_tensor("onesb", [128, 1], f32, kind="ExternalInput").ap()
    iden_d = nc.dram_tensor("iden", [128, 128], f16, kind="ExternalInput").ap()
    wrm_d = nc.dram_tensor("wrm", [128, 16], f16, kind="ExternalInput").ap()
    part_d = nc.dram_tensor("part", [LI, HID], f16, kind="ExternalOutput").ap()

    x2t_r = x2t_d.rearrange("(n p) i -> p n i", p=128)   # [128, 18, 2048]
    wq_r = wq_d.rearrange("(n p) d -> p n d", p=128)     # [128, 18, 256]
    wkv_r = wkv_d.rearrange("(n p) d -> p n d", p=128)
    wo_r = wo_d.rearrange("(n p) h -> p n h", p=128)     # [128, 2, 2304]

    TANH = mybir.ActivationFunctionType.Tanh
    EXP = mybir.ActivationFunctionType.Exp
    COPY = mybir.ActivationFunctionType.Copy

    with tile.TileContext(nc) as tc:
        with (
            tc.tile_pool(name="const", bufs=1) as cpool,
            tc.tile_pool(name="kv", bufs=1) as kvpool,
            tc.tile_pool(name="kvs", bufs=2) as kvspool,
            tc.tile_pool(name="qs", bufs=2) as qpool,
            tc.tile_pool(name="th", bufs=5) as thpool,
            tc.tile_pool(name="pp", bufs=6) as ppool,
            tc.tile_pool(name="ob", bufs=2) as obpool,
            tc.tile_pool(name="os", bufs=4) as ospool,
            tc.tile_pool(name="ac", bufs=2) as acpool,
            tc.tile_pool(name="ri", bufs=2) as ripool,
            tc.tile_pool(name="dram", bufs=1, space="DRAM") as dram,
            tc.tile_pool(name="pq", bufs=2, space="PSUM") as pq,
            tc.tile_pool(name="pa", bufs=2, space="PSUM") as pa,
            tc.tile_pool(name="po", bufs=2, space="PSUM") as po,
            tc.tile_pool(name="pt", bufs=2, space="PSUM") as pt,
        ):
            # DRAM bounce buffers for the pair AllGathers (one per i-block)
            kv_bi = dram.tile([NIB, 128, 2, 512], f16)
            kv_bo = dram.tile([NIB, 2, 128, 2, 512], f16)
            wrm_bi = dram.tile([128, 16], f16)
            wrm_bo = dram.tile([2, 128, 16], f16)

            # scratch for PE warmup + act-table warmup (zeros); memset goes
            # FIRST on the gpsimd queue so the PE warmup isn't stuck behind
            # the warmup-collective enqueue
            scratch = cpool.tile([128, 640], f16, tag="scratch")
            nc.gpsimd.memset(scratch[:, :], 0.0)

            # warmup collective: absorbs the one-time CC rendezvous latency
            nc.sync.dma_start(out=wrm_bi[:, :], in_=wrm_d)
            nc.gpsimd.collective_compute(
                "AllGather",
                mybir.AluOpType.bypass,
                replica_groups=PAIRS,
                ins=[wrm_bi[:, :].opt()],
                outs=[wrm_bo[:, :, :].opt()],
            )

            warm = thpool.tile([128, 8], f32, tag="th", name="warm")
            nc.scalar.activation(warm[:, :], scratch[:, 0:8], TANH,
                                 scale=SCALE / SOFTCAP)
            nc.scalar.activation(warm[:, :], warm[:, :], EXP, scale=SOFTCAP)
            # spin the PE up to full clock before the first real matmul
            for w in range(22):
                wp = pq.tile([128, 512], f32, tag="pq", name="wp")
                nc.tensor.matmul(
                    wp[:, :], scratch[:, 0:128], scratch[:, 128:640],
                    start=True, stop=True,
                )

            # ---- resident loads, spread across four DGE rings (sync,
            # vector, gpsimd, scalar) so startup bandwidth isn't capped by a
            # single ring's ~290 GB/s ----
            x2t = cpool.tile([128, NCC, LI], f16, tag="x2t")
            wkv = cpool.tile([128, NCC, D], f16, tag="wkv")
            wq = cpool.tile([128, NCC, D], f16, tag="wq")
            # critical path: first kv-projection; split the first chunks across
            # queues so the first matmuls can start as early as possible
            for q4 in range(4):
                nc.sync.dma_start(
                    out=x2t[:, 0, q4 * 128:(q4 + 1) * 128],
                    in_=x2t_r[:, 0, q4 * 128:(q4 + 1) * 128],
                )
            for h2 in range(2):
                hsl = slice(h2 * 128, (h2 + 1) * 128)
                nc.sync.dma_start(out=wkv[:, 0, hsl], in_=wkv_r[:, 0, hsl])
            for cc in range(1, NCC):
                nc.sync.dma_start(out=x2t[:, cc, 0:512], in_=x2t_r[:, cc, 0:512])
                nc.sync.dma_start(out=wkv[:, cc, :], in_=wkv_r[:, cc, :])
            # gpsimd ring (idle between collectives): rope tables (needed for
            # q0 rope ~20us in) + x2t block 2
            cos = cpool.tile([128, LI], f16, tag="cos")
            sin = cpool.tile([128, LI], f16, tag="sin")
            nc.gpsimd.dma_start(out=cos[:, :], in_=cos_d)
            nc.gpsimd.dma_start(out=sin[:, :], in_=sin_d)
            nc.gpsimd.dma_start(out=x2t[:, :, 1024:1536], in_=x2t_r[:, :, 1024:1536])
            # sync ring continues, in order of first use: q weights (fused,
            # needed ~16us), x2t block 1 (~24us), then late-needed consts
            nc.sync.dma_start(out=wq[:, :, :], in_=wq_r)
            nc.sync.dma_start(out=x2t[:, :, 512:1024], in_=x2t_r[:, :, 512:1024])
            iden = cpool.tile([128, 128], f16, tag="iden")
            nc.sync.dma_start(out=iden[:, :], in_=iden_d)
            tri = cpool.tile([128, 2048], bf16, tag="tri")
            nc.sync.dma_start(out=tri[:, :], in_=tri_d)
            onesb = cpool.tile([128, 1], f32, tag="onesb")
            nc.sync.dma_start(out=onesb[:, :], in_=onesb_d)
            wo = cpool.tile([128, 2, HID], f16, tag="wo")

            # per-i-block persistent K^T (fp16, [d_chunk, j]) and V (bf16, [j, d])
            kts = [
                kvpool.tile([128, 2, 512], f16, tag=f"kt{b}", name=f"kt{b}")
                for b in range(NIB)
            ]
            vts = [
                kvpool.tile([128, 4, D], bf16, tag=f"vt{b}", name=f"vt{b}")
                for b in range(NIB)
            ]

            qsbs = [
                qpool.tile([128, 2, 512], f16, tag=f"qsb{b}", name=f"qsb{b}")
                for b in range(NIB)
            ]

            def rope_sb(c0, c1, out0, out1, isl):
                # out0 = c0*cos - c1*sin ; out1 = c1*cos + c0*sin
                # (cos/sin identical for both 128-row halves of the head dim)
                for dst, a, b_, op in ((0, c0, c1, "sub"), (1, c1, c0, "add")):
                    ta = thpool.tile([128, 512], f32, tag="th", name="ta")
                    nc.vector.tensor_mul(ta[:, :], a, cos[:, isl])
                    tb = thpool.tile([128, 512], f32, tag="th", name="tb")
                    nc.vector.tensor_mul(tb[:, :], b_, sin[:, isl])
                    dstap = out0 if dst == 0 else out1
                    if op == "sub":
                        nc.vector.tensor_sub(dstap, ta[:, :], tb[:, :])
                    else:
                        nc.vector.tensor_add(dstap, ta[:, :], tb[:, :])

            def proj(wt, dst_sb):
                # dst_sb[:, dc, :] = (wt.T @ x2t_block) via scalar PSUM drain
                for dc in range(2):
                    pp_ = pq.tile([128, 512], f32, tag="pq", name="prj")
                    for cc in range(NCC):
                        nc.tensor.matmul(
                            pp_[:, :],
                            wt[:, cc, dc * 128:(dc + 1) * 128],
                            x2t[:, cc, dst_sb[1]],
                            start=(cc == 0),
                            stop=(cc == NCC - 1),
                        )
                    nc.scalar.activation(dst_sb[0][:, dc, :], pp_[:, :], COPY)

            # ===== phase 1: kv blocks front-loaded so the pair AllGathers
            # chain right behind the warmup rendezvous; q blocks fill in =====
            # kv(b): K^T on even cores / V^T on odd (pre-rope), -> AllGather b
            # q(b):  this head's Q block + rope
            def kv_block(b):
                isl = slice(b * 512, (b + 1) * 512)
                kvt = kvspool.tile([128, 2, 512], f16, tag="kvt", name="kvt")
                proj(wkv, (kvt, isl))
                # bounce write on the scalar DGE ring: the data was just
                # produced by scalar copies, and the sync ring is backlogged
                # with the resident-load stream
                nc.scalar.dma_start(out=kv_bi[b, :, :, :], in_=kvt[:, :, :])
                nc.gpsimd.collective_compute(
                    "AllGather",
                    mybir.AluOpType.bypass,
                    replica_groups=PAIRS,
                    ins=[kv_bi[b, :, :, :].opt()],
                    outs=[kv_bo[b, :, :, :, :].opt()],
                )

            def q_block(b):
                isl = slice(b * 512, (b + 1) * 512)
                qc = kvspool.tile([128, 2, 512], f16, tag="qc", name="qc")
                proj(wq, (qc, isl))
                qsb = qsbs[b]
                rope_sb(qc[:, 0, :], qc[:, 1, :], qsb[:, 0, :], qsb[:, 1, :], isl)

            kv_block(0)
            # scalar ring: block 3 + wo issue after the kv0 bounce write so
            # the first collective isn't queued behind 3.5MB of transfers
            nc.scalar.dma_start(out=x2t[:, :, 1536:2048], in_=x2t_r[:, :, 1536:2048])
            nc.scalar.dma_start(out=wo[:, :, :], in_=wo_r)
            q_block(0)
            kv_block(1)
            q_block(1)
            kv_block(2)
            kv_block(3)
            q_block(2)
            q_block(3)

            # lazy unpack: block b's K^T rope / V^T transposes are only needed
            # near the END of phase-2 block b, so they interleave into phase 2
            flip = [0]
            vtts = {}

            def unpack_load(b):
                # DRAM->SBUF loads + K rope (vector); gated on collective b
                krsb = kvspool.tile([128, 2, 512], f16, tag="kr", name=f"kr{b}")
                nc.sync.dma_start(out=krsb[:, :, :], in_=kv_bo[b, 0, :, :, :])
                psl = slice(b * 512, (b + 1) * 512)
                rope_sb(krsb[:, 0, :], krsb[:, 1, :],
                        kts[b][:, 0, :], kts[b][:, 1, :], psl)
                vtt = kvspool.tile([128, 2, 512], f16, tag="vtt", name=f"vtt{b}")
                nc.sync.dma_start(out=vtt[:, :, :], in_=kv_bo[b, 1, :, :, :])
                vtts[b] = vtt

            def unpack_transpose(jb):
                vtt = vtts.pop(jb)
                for js in range(4):
                    tp = pt.tile([128, 256], f16, tag="pt", name="tp")
                    for dc in range(2):
                        nc.tensor.transpose(
                            tp[:, dc * 128:(dc + 1) * 128],
                            vtt[:, dc, js * 128:(js + 1) * 128],
                            iden[:, :],
                        )
                    dst = vts[jb][:, js, :]
                    if flip[0] % 2 == 0:
                        nc.scalar.activation(dst, tp[:, :], COPY)
                    else:
                        nc.vector.tensor_copy(out=dst, in_=tp[:, :])
                    flip[0] += 1

            unpack_load(0)
            unpack_transpose(0)

            # ===== phase 2: attention + output projection, software-pipelined =====
            mflip = [0]
            dflip = [0]

            def emit_norm_item(osb, rinv, ib, isub, hc, hw, outs, last):
                outp = pq.tile([128, hw], f32, tag="pq", name="outp")
                for dc in range(2):
                    nc.tensor.matmul(
                        outp[:, :],
                        osb[:, dc, isub * 128:(isub + 1) * 128],
                        wo[:, dc, hc:hc + hw],
                        start=(dc == 0),
                        stop=(dc == 1),
                    )
                # PSUM drain + 1/den scale, alternating vector/scalar so
                # neither engine paces the outp ring
                if mflip[0] % 2 == 0:
                    nc.vector.tensor_scalar_mul(
                        outs[:, hc:hc + hw], outp[:, :], rinv[:, isub:isub + 1]
                    )
                else:
                    nc.scalar.mul(outs[:, hc:hc + hw], outp[:, :],
                                  rinv[:, isub:isub + 1])
                mflip[0] += 1
                if last:
                    # one consolidated 576KB write per 128-row slab, rings
                    # alternated to halve the drain time of the final block
                    eng = nc.sync if dflip[0] % 2 == 0 else nc.scalar
                    dflip[0] += 1
                    eng.dma_start(
                        out=part_d[ib * 512 + isub * 128:
                                   ib * 512 + (isub + 1) * 128, :],
                        in_=outs[:, :],
                    )

            def make_den(ops, acc, ib):
                # transposed denominator: 4 K=128 ones-matmuls -> [128, 4]
                # (borrowing an sp-shaped PSUM slot), then one wide reciprocal
                den = pa.tile([128, 512], f32, tag="pa", name="den")
                for s in range(4):
                    nc.tensor.matmul(
                        den[:, s:s + 1],
                        acc[:, s * 128:(s + 1) * 128],
                        onesb[:, :],
                        start=True,
                        stop=True,
                    )
                rinv = ripool.tile([128, 4], f32, tag="ri", name="rinv")
                nc.vector.reciprocal(rinv[:, :], den[:, 0:4])
                osb = obpool.tile([128, 2, 512], f16, tag="osb", name="osb")
                for dc in range(2):
                    nc.vector.tensor_copy(out=osb[:, dc, :], in_=ops[dc][:, :])
                items = []
                chunks = _hid_chunks()
                for isub in range(4):
                    outs = ospool.tile([128, HID], f16, tag="os", name="outs")
                    for ci, (hc, hw) in enumerate(chunks):
                        items.append((osb, rinv, ib, isub, hc, hw, outs,
                                      ci == len(chunks) - 1))
                return items

            pend = []
            prev = None
            for ib in range(NIB):
                qsb = qsbs[ib]
                njc = 4 * ib + 4
                ops = [
                    po.tile([128, 512], f32, tag="po", name="op0"),
                    po.tile([128, 512], f32, tag="po", name="op1"),
                ]
                acc = acpool.tile([128, 512], f32, tag="ac", name="acc")
                pbuf = []

                def av(jc):
                    jb, js = jc // 4, jc % 4
                    first, last = (jc == 0), (jc == njc - 1)
                    for dc in range(2):
                        nc.tensor.matmul(
                            ops[dc][:, :],
                            vts[jb][:, js, dc * 128:(dc + 1) * 128],
                            pbuf[jc][:, :],
                            start=first,
                            stop=last,
                        )

                for jc in range(njc):
                    jb, js = jc // 4, jc % 4
                    sp = pa.tile([128, 512], f32, tag="pa", name="sp")
                    for dc in range(2):
                        nc.tensor.matmul(
                            sp[:, :],
                            kts[jb][:, dc, js * 128:(js + 1) * 128],
                            qsb[:, dc, :],
                            start=(dc == 0),
                            stop=(dc == 1),
                        )
                    th = thpool.tile([128, 512], f32, tag="th", name="th")
                    nc.scalar.activation(th[:, :], sp[:, :], TANH, scale=SCALE / SOFTCAP)
                    p = ppool.tile([128, 512], bf16, tag="pp", name="p")
                    nc.scalar.activation(p[:, :], th[:, :], EXP, scale=SOFTCAP)
                    if jb == ib:  # diagonal block: causal mask via 0/1 multiply
                        pm = ppool.tile([128, 512], bf16, tag="pp", name="pm")
                        nc.vector.tensor_mul(
                            pm[:, :], p[:, :], tri[:, js * 512:(js + 1) * 512]
                        )
                        p = pm
                    pbuf.append(p)
                    # denominator accumulates on the otherwise-idle gpsimd
                    if jc == 0:
                        nc.gpsimd.tensor_copy(out=acc[:, :], in_=p[:, :])
                    else:
                        nc.gpsimd.tensor_add(acc[:, :], acc[:, :], p[:, :])
                    # previous block's denominator at jc==0 (its gpsimd chain
                    # has drained by now), wo projection spread from jc==1;
                    # this block's V transposes slot in at jc==2
                    if jc == 0 and prev is not None:
                        pend = make_den(*prev)
                        prev = None
                    if jc == 2 and ib in vtts:
                        unpack_transpose(ib)
                    if jc >= 1:
                        for _ in range(3):
                            if pend:
                                emit_norm_item(*pend.pop(0))
                    if jc >= 3:
                        av(jc - 3)
                av(njc - 3)
                av(njc - 2)
                av(njc - 1)
                while pend:
                    emit_norm_item(*pend.pop(0))
                prev = (ops, acc, ib)
                if ib + 1 < NIB:
                    unpack_load(ib + 1)
            pend = make_den(*prev)
            while pend:
                emit_norm_item(*pend.pop(0))
    nc.compile()
    return nc


def _host_prep(x, wq, wk, wv, wo):
    """Build per-core input maps (head h on core h; pair 2g/2g+1 shares kv g:
    even core carries wk, odd carries wv)."""
    x2 = x[0, LI:, :]                                   # [2048, 2304]
    x2t = np.ascontiguousarray(x2.T).astype(np.float16)  # [2304, 2048]

    inv_freq = 1.0 / (ROPE_BASE ** (np.arange(0, D, 2, dtype=np.float32) / D))
    t = np.arange(LI, L, dtype=np.float32)
    freqs = np.outer(t, inv_freq)                        # [2048, 128]
    # both 128-row halves of the full [2048, 256] table are identical; the
    # device shares one copy
    cost = np.ascontiguousarray(np.cos(freqs).astype(np.float32).T).astype(np.float16)
    sint = np.ascontiguousarray(np.sin(freqs).astype(np.float32).T).astype(np.float16)

    tri = np.zeros((128, 2048), dtype=_BF16)
    jj = np.arange(128)[:, None]
    ii = np.arange(512)[None, :]
    for k in range(4):
        tri[:, k * 512:(k + 1) * 512] = (128 * k + jj <= ii).astype(_BF16)

    onesb = np.ones((128, 1), dtype=np.float32)
    iden = np.eye(128, dtype=np.float16)
    wrm = np.zeros((128, 16), dtype=np.float16)

    in_maps = []
    for h in range(H):
        g = h // 2
        wkv_src = wk if h % 2 == 0 else wv
        in_maps.append({
            "x2t": x2t,
            "wq": np.ascontiguousarray(wq[:, h * D:(h + 1) * D]).astype(np.float16),
            "wkv": np.ascontiguousarray(
                wkv_src[:, g * D:(g + 1) * D]).astype(np.float16),
            "wo": np.ascontiguousarray(wo[h * D:(h + 1) * D, :]).astype(np.float16),
            "cost": cost,
            "sint": sint,
            "tri": tri,
            "onesb": onesb,
            "iden": iden,
            "wrm": wrm,
        })
    return in_maps


def _first_half_row(x, wv, wo):
    """Rows 0..2047 of the output: uniform attention over all 4096 keys."""
    vmean = x[0].mean(axis=0, dtype=np.float64).astype(np.float32) @ wv  # [1024]
    per_kv = vmean.reshape(HKV, D)
    o = np.concatenate([per_kv[h // 2] for h in range(H)])  # [2048]
    return o @ wo                                           # [2304]


def _mask_is_causal(mask):
    m = mask[0, 0]
    causal = np.triu(np.full((L, L), np.float32(NEG), dtype=np.float32), k=1)
    return np.array_equal(m, causal)


def _numpy_fallback(x, mask, wq, wk, wv, wo):
    """Direct fp32 replication of the reference (only used if mask is unusual)."""
    xb = x[0]
    q = (xb @ wq).reshape(L, H, D)
    k = (xb @ wk).reshape(L, HKV, D)
    v = (xb @ wv).reshape(L, HKV, D)
    inv_freq = 1.0 / (ROPE_BASE ** (np.arange(0, D, 2, dtype=np.float32) / D))
    t = np.arange(L, dtype=np.float32)
    emb = np.concatenate([np.outer(t, inv_freq)] * 2, axis=-1)
    cos = np.cos(emb).astype(np.float32)[:, None, :]
    sin = np.sin(emb).astype(np.float32)[:, None, :]

    def rope(a):
        a1, a2 = a[..., :D // 2], a[..., D // 2:]
        return a * cos + np.concatenate([-a2, a1], axis=-1) * sin

    q, k = rope(q), rope(k)
    col_keep = np.arange(L) >= (L - 2048)
    out = np.zeros((L, H * D), dtype=np.float32)
    for h in range(H):
        g = h // 2
        s = (q[:, h] @ k[:, g].T) * np.float32(SCALE)
        s = np.float32(SOFTCAP) * np.tanh(s / np.float32(SOFTCAP))
        s = s + mask[0, 0]
        s = np.where(col_keep[None, :], s, np.float32(NEG))
        s = s - s.max(axis=1, keepdims=True)
        p = np.exp(s)
        p /= p.sum(axis=1, keepdims=True)
        out[:, h * D:(h + 1) * D] = p @ v[:, g]
    return (out @ wo).reshape(1, L, HID)


def _run_device(in_maps, trace=False, trace_cores=None):
    from concourse.bass_utils import run_bass_kernel_spmd

    if "nc" not in _CACHE:
        _CACHE["nc"] = _build_nc()
    nc = _CACHE["nc"]
    return run_bass_kernel_spmd(
        nc, in_maps, list(range(H)), trace=trace, trace_cores=trace_cores
    )


def kernel(x, mask, wq, wk, wv, wo):
    x = np.asarray(x, dtype=np.float32)
    mask = np.asarray(mask, dtype=np.float32)
    wq = np.asarray(wq, dtype=np.float32)
    wk = np.asarray(wk, dtype=np.float32)
    wv = np.asarray(wv, dtype=np.float32)
    wo = np.asarray(wo, dtype=np.float32)

    if not _mask_is_causal(mask):
        return _numpy_fallback(x, mask, wq, wk, wv, wo)

    in_maps = _host_prep(x, wq, wk, wv, wo)
    res = _run_device(in_maps)
    parts = np.zeros((LI, HID), dtype=np.float32)
    for c in range(H):
        parts += res.results[c]["part"].astype(np.float32)

    out = np.empty((1, L, HID), dtype=np.float32)
    out[0, :LI, :] = _first_half_row(x, wv, wo)[None, :]
    out[0, LI:, :] = parts
    return out



# revision 18
# speedup vs baseline: 1.0386x; 1.0386x over previous
"""Gemma2 sliding-window attention (B=1, L=4096, H=8/KV4, D=256, HID=2304, W=2048)
on 8 TRN2 NeuronCores via Bass/Tile.

Key structural facts of the reference (validated against it numerically):
- The window mask keeps only key columns >= 2048 for ALL rows; combined with
  the causal mask, rows < 2048 end up with every logit == -1e9 exactly in fp32
  (|softcapped score| < 32 < ulp(1e9)/2), so softmax is uniform over all 4096
  keys: rows 0..2047 of the output are one constant row = colmean(v) @ wo.
- Rows >= 2048 are standard causal softcapped attention over keys [2048, i];
  the -1e9 terms underflow to exactly 0 in the fp32 softmax.
- Softcap bounds logits to [-50, 50], so exp() without max-subtraction is safe
  in fp32 and matches the reference softmax up to rounding.

Sharding: one query head per core. The K/V projections for kv head g=h//2 are
deduplicated across the pair (2g, 2g+1): the even core projects K^T, the odd
core projects V^T (identical SPMD programs; only the per-core weight data
differ), exchanged per 512-block with on-device pair AllGathers (a tiny warmup
collective at kernel start absorbs the ~30us one-time rendezvous). Both cores
then rope the gathered K^T (slot 0) with the shared rope tables and transpose
V^T (slot 1) into V [j, d] layout with PE transposes. kv/q projection blocks
are interleaved so the x2t input stream paces evenly, and warmup matmuls on
scratch data bring the PE to full clock before real work.

Phase 2 computes scores in [j_part, i_free] layout, unnormalized oT in PSUM,
projects through this head's wo slice (the previous block's 20 projection
chunks are spread 3-per-score-chunk through the loop to keep every queue
shallow), and normalizes per query row with a per-partition 1/denominator
scale fused into the PSUM->SBUF drain, alternating vector/scalar. The
denominator accumulates on gpsimd and collapses to a transposed [128, 4]
vector with four K=128 ones-matmuls per i-block. The gathered K/V blocks
unpack lazily inside phase 2 (block b is only needed near the end of phase-2
block b), hiding the collective latency. fp16 partials [2048, 2304] stream
out; host sums the 8 partials in fp32 and prepends the constant first-half
row.
"""
import sys

sys.path.insert(0, "/opt/trn_rl_repo")

import numpy as np
import ml_dtypes

H = 8
HKV = 4
D = 256
HID = 2304
L = 4096
LI = 2048          # second-half rows (local)
NCC = HID // 128   # 18 contraction chunks
NIB = LI // 512    # 4 i-blocks of 512
SCALE = (HID // H) ** -0.5
SOFTCAP = 50.0
NEG = -1e9
ROPE_BASE = 10000.0

_BF16 = ml_dtypes.bfloat16

_CACHE = {}

PAIRS = [[0, 1], [2, 3], [4, 5], [6, 7]]


def _hid_chunks():
    out = []
    c = 0
    while c < HID:
        w = min(512, HID - c)
        out.append((c, w))
        c += w
    return out


def _build_nc():
    import concourse.bass as bass
    import concourse.mybir as mybir
    import concourse.tile as tile
    from concourse import bacc

    f32 = mybir.dt.float32
    f16 = mybir.dt.float16
    bf16 = mybir.dt.bfloat16

    nc = bacc.Bacc("TRN2", target_bir_lowering=False, debug=False, num_devices=8)

    # weights arrive pre-tiled in SBUF layout so each loads with one DMA of
    # 4KB+ per-partition lines (~370 GB/s vs ~15 GB/s for 256B lines)
    x2t_d = nc.dram_tensor("x2t", [HID, LI], f16, kind="ExternalInput").ap()
    wq_d = nc.dram_tensor("wq", [128, NCC, D], f16, kind="ExternalInput").ap()
    wkv_d = nc.dram_tensor("wkv", [128, NCC, D], f16, kind="ExternalInput").ap()
    wo_d = nc.dram_tensor("wo", [128, 2, HID], f16, kind="ExternalInput").ap()
    # rope tables: rows d and d+128 of the [D, LI] table are identical, so
    # only the first 128 rows are stored and shared by both halves
    cos_d = nc.dram_tensor("cost", [128, LI], f16, kind="ExternalInput").ap()
    sin_d = nc.dram_tensor("sint", [128, LI], f16, kind="ExternalInput").ap()
    tri_d = nc.dram_tensor("tri", [128, 2048], bf16, kind="ExternalInput").ap()
    onesb_d = nc.dram_tensor("onesb", [128, 1], f32, kind="ExternalInput").ap()
    iden_d = nc.dram_tensor("iden", [128, 128], f16, kind="ExternalInput").ap()
    wrm_d = nc.dram_tensor("wrm", [128, 16], f16, kind="ExternalInput").ap()
    part_d = nc.dram_tensor("part", [LI, HID], f16, kind="ExternalOutput").ap()

    x2t_r = x2t_d.rearrange("(n p) i -> p n i", p=128)   # [128, 18, 2048]

    TANH = mybir.ActivationFunctionType.Tanh
    EXP = mybir.ActivationFunctionType.Exp
    COPY = mybir.ActivationFunctionType.Copy

    with tile.TileContext(nc) as tc:
        with (
            tc.tile_pool(name="const", bufs=1) as cpool,
            tc.tile_pool(name="kv", bufs=1) as kvpool,
            tc.tile_pool(name="kvs", bufs=2) as kvspool,
            tc.tile_pool(name="qs", bufs=2) as qpool,
            tc.tile_pool(name="th", bufs=5) as thpool,
            tc.tile_pool(name="pp", bufs=6) as ppool,
            tc.tile_pool(name="ob", bufs=2) as obpool,
            tc.tile_pool(name="os", bufs=6) as ospool,
            tc.tile_pool(name="ac", bufs=2) as acpool,
            tc.tile_pool(name="ri", bufs=2) as ripool,
            tc.tile_pool(name="dram", bufs=1, space="DRAM") as dram,
            tc.tile_pool(name="pq", bufs=2, space="PSUM") as pq,
            tc.tile_pool(name="pa", bufs=2, space="PSUM") as pa,
            tc.tile_pool(name="po", bufs=2, space="PSUM") as po,
            tc.tile_pool(name="pt", bufs=2, space="PSUM") as pt,
        ):
            # DRAM bounce buffers for the pair AllGathers (one per i-block)
            kv_bi = dram.tile([NIB, 128, 2, 512], f16)
            kv_bo = dram.tile([NIB, 2, 128, 2, 512], f16)
            wrm_bi = dram.tile([128, 16], f16)
            wrm_bo = dram.tile([2, 128, 16], f16)

            # scratch for PE warmup + act-table warmup (zeros); memset goes
            # FIRST on the gpsimd queue so the PE warmup isn't stuck behind
            # the warmup-collective enqueue
            scratch = cpool.tile([128, 640], f16, tag="scratch")
            nc.gpsimd.memset(scratch[:, :], 0.0)

            # warmup collective: absorbs the one-time CC rendezvous latency
            nc.sync.dma_start(out=wrm_bi[:, :], in_=wrm_d)
            nc.gpsimd.collective_compute(
                "AllGather",
                mybir.AluOpType.bypass,
                replica_groups=PAIRS,
                ins=[wrm_bi[:, :].opt()],
                outs=[wrm_bo[:, :, :].opt()],
            )

            warm = thpool.tile([128, 8], f32, tag="th", name="warm")
            nc.scalar.activation(warm[:, :], scratch[:, 0:8], TANH,
                                 scale=SCALE / SOFTCAP)
            nc.scalar.activation(warm[:, :], warm[:, :], EXP, scale=SOFTCAP)
            # spin the PE up to full clock before the first real matmul
            for w in range(22):
                wp = pq.tile([128, 512], f32, tag="pq", name="wp")
                nc.tensor.matmul(
                    wp[:, :], scratch[:, 0:128], scratch[:, 128:640],
                    start=True, stop=True,
                )

            # ---- resident loads, spread across the three DGE rings (sync,
            # scalar, gpsimd) with fat per-partition lines ----
            x2t = cpool.tile([128, NCC, LI], f16, tag="x2t")
            wkv = cpool.tile([128, NCC, D], f16, tag="wkv")
            wq = cpool.tile([128, NCC, D], f16, tag="wq")
            # scalar ring: pre-tiled weights, one fast DMA each, in order of
            # first use (wkv immediately, wq ~8us, wo at first norm item)
            nc.scalar.dma_start(out=wkv[:, :, :], in_=wkv_d)
            nc.scalar.dma_start(out=wq[:, :, :], in_=wq_d)
            wo = cpool.tile([128, 2, HID], f16, tag="wo")
            nc.scalar.dma_start(out=wo[:, :, :], in_=wo_d)
            # sync ring: i-block 0 of x2t per contraction chunk (feeds the
            # first kv/q projections chunk by chunk)
            for cc in range(NCC):
                nc.sync.dma_start(out=x2t[:, cc, 0:512], in_=x2t_r[:, cc, 0:512])
            # gpsimd ring (idle between collectives): rope tables (needed for
            # q0 rope ~15us in), then i-blocks 1-3 per chunk with 3KB lines
            cos = cpool.tile([128, LI], f16, tag="cos")
            sin = cpool.tile([128, LI], f16, tag="sin")
            nc.gpsimd.dma_start(out=cos[:, :], in_=cos_d)
            nc.gpsimd.dma_start(out=sin[:, :], in_=sin_d)
            for cc in range(NCC):
                nc.gpsimd.dma_start(
                    out=x2t[:, cc, 512:2048], in_=x2t_r[:, cc, 512:2048]
                )
            # sync ring continues with late-needed consts
            iden = cpool.tile([128, 128], f16, tag="iden")
            nc.sync.dma_start(out=iden[:, :], in_=iden_d)
            tri = cpool.tile([128, 2048], bf16, tag="tri")
            nc.sync.dma_start(out=tri[:, :], in_=tri_d)
            onesb = cpool.tile([128, 1], f32, tag="onesb")
            nc.sync.dma_start(out=onesb[:, :], in_=onesb_d)

            # per-i-block persistent K^T (fp16, [d_chunk, j]) and V (bf16, [j, d])
            kts = [
                kvpool.tile([128, 2, 512], f16, tag=f"kt{b}", name=f"kt{b}")
                for b in range(NIB)
            ]
            vts = [
                kvpool.tile([128, 4, D], bf16, tag=f"vt{b}", name=f"vt{b}")
                for b in range(NIB)
            ]

            qsbs = [
                qpool.tile([128, 2, 512], f16, tag=f"qsb{b}", name=f"qsb{b}")
                for b in range(NIB)
            ]

            def rope_sb(c0, c1, out0, out1, isl):
                # out0 = c0*cos - c1*sin ; out1 = c1*cos + c0*sin
                # (cos/sin identical for both 128-row halves of the head dim)
                for dst, a, b_, op in ((0, c0, c1, "sub"), (1, c1, c0, "add")):
                    ta = thpool.tile([128, 512], f32, tag="th", name="ta")
                    nc.vector.tensor_mul(ta[:, :], a, cos[:, isl])
                    tb = thpool.tile([128, 512], f32, tag="th", name="tb")
                    nc.vector.tensor_mul(tb[:, :], b_, sin[:, isl])
                    dstap = out0 if dst == 0 else out1
                    if op == "sub":
                        nc.vector.tensor_sub(dstap, ta[:, :], tb[:, :])
                    else:
                        nc.vector.tensor_add(dstap, ta[:, :], tb[:, :])

            def proj(wt, dst_sb):
                # dst_sb[:, dc, :] = (wt.T @ x2t_block) via scalar PSUM drain
                for dc in range(2):
                    pp_ = pq.tile([128, 512], f32, tag="pq", name="prj")
                    for cc in range(NCC):
                        nc.tensor.matmul(
                            pp_[:, :],
                            wt[:, cc, dc * 128:(dc + 1) * 128],
                            x2t[:, cc, dst_sb[1]],
                            start=(cc == 0),
                            stop=(cc == NCC - 1),
                        )
                    nc.scalar.activation(dst_sb[0][:, dc, :], pp_[:, :], COPY)

            # ===== phase 1: kv blocks front-loaded so the pair AllGathers
            # chain right behind the warmup rendezvous; q blocks fill in =====
            # kv(b): K^T on even cores / V^T on odd (pre-rope), -> AllGather b
            # q(b):  this head's Q block + rope
            def kv_block(b):
                isl = slice(b * 512, (b + 1) * 512)
                kvt = kvspool.tile([128, 2, 512], f16, tag="kvt", name="kvt")
                proj(wkv, (kvt, isl))
                # bounce write on the scalar DGE ring: the data was just
                # produced by scalar copies, and the sync ring is backlogged
                # with the resident-load stream
                nc.scalar.dma_start(out=kv_bi[b, :, :, :], in_=kvt[:, :, :])
                nc.gpsimd.collective_compute(
                    "AllGather",
                    mybir.AluOpType.bypass,
                    replica_groups=PAIRS,
                    ins=[kv_bi[b, :, :, :].opt()],
                    outs=[kv_bo[b, :, :, :, :].opt()],
                )

            def q_block(b):
                isl = slice(b * 512, (b + 1) * 512)
                qc = kvspool.tile([128, 2, 512], f16, tag="qc", name="qc")
                proj(wq, (qc, isl))
                qsb = qsbs[b]
                rope_sb(qc[:, 0, :], qc[:, 1, :], qsb[:, 0, :], qsb[:, 1, :], isl)

            kv_block(0)
            q_block(0)
            kv_block(1)
            q_block(1)
            kv_block(2)
            kv_block(3)
            q_block(2)
            q_block(3)

            # lazy unpack: block b's K^T rope / V^T transposes are only needed
            # near the END of phase-2 block b, so they interleave into phase 2
            flip = [0]
            vtts = {}

            def unpack_load(b):
                # DRAM->SBUF loads + K rope (vector); gated on collective b
                krsb = kvspool.tile([128, 2, 512], f16, tag="kr", name=f"kr{b}")
                nc.sync.dma_start(out=krsb[:, :, :], in_=kv_bo[b, 0, :, :, :])
                psl = slice(b * 512, (b + 1) * 512)
                rope_sb(krsb[:, 0, :], krsb[:, 1, :],
                        kts[b][:, 0, :], kts[b][:, 1, :], psl)
                vtt = kvspool.tile([128, 2, 512], f16, tag="vtt", name=f"vtt{b}")
                nc.sync.dma_start(out=vtt[:, :, :], in_=kv_bo[b, 1, :, :, :])
                vtts[b] = vtt

            def unpack_transpose(jb):
                vtt = vtts.pop(jb)
                for js in range(4):
                    tp = pt.tile([128, 256], f16, tag="pt", name="tp")
                    for dc in range(2):
                        nc.tensor.transpose(
                            tp[:, dc * 128:(dc + 1) * 128],
                            vtt[:, dc, js * 128:(js + 1) * 128],
                            iden[:, :],
                        )
                    dst = vts[jb][:, js, :]
                    if flip[0] % 2 == 0:
                        nc.scalar.activation(dst, tp[:, :], COPY)
                    else:
                        nc.vector.tensor_copy(out=dst, in_=tp[:, :])
                    flip[0] += 1

            unpack_load(0)
            unpack_transpose(0)

            # ===== phase 2: attention + output projection, software-pipelined =====
            mflip = [0]
            dflip = [0]

            def emit_norm_item(osb, rinv, ib, isub, hc, hw, outs, oc, flush,
                               last_blk):
                outp = pq.tile([128, hw], f32, tag="pq", name="outp")
                for dc in range(2):
                    nc.tensor.matmul(
                        outp[:, :],
                        osb[:, dc, isub * 128:(isub + 1) * 128],
                        wo[:, dc, hc:hc + hw],
                        start=(dc == 0),
                        stop=(dc == 1),
                    )
                # PSUM drain + 1/den scale, alternating vector/scalar so
                # neither engine paces the outp ring
                if mflip[0] % 2 == 0:
                    nc.vector.tensor_scalar_mul(
                        outs[:, oc:oc + hw], outp[:, :], rinv[:, isub:isub + 1]
                    )
                else:
                    nc.scalar.mul(outs[:, oc:oc + hw], outp[:, :],
                                  rinv[:, isub:isub + 1])
                mflip[0] += 1
                if flush:
                    # write the pair of chunks (2KB per-partition lines); the
                    # final i-block alternates rings so its tail drains 2x
                    if last_blk:
                        eng = nc.sync if dflip[0] % 2 == 0 else nc.scalar
                        dflip[0] += 1
                    else:
                        eng = nc.sync
                    w = oc + hw              # total cols in this outs tile
                    base = hc - oc           # output col of outs[:, 0]
                    eng.dma_start(
                        out=part_d[ib * 512 + isub * 128:
                                   ib * 512 + (isub + 1) * 128,
                                   base:base + w],
                        in_=outs[:, :w],
                    )

            def make_den(ops, acc, ib):
                # transposed denominator: 4 K=128 ones-matmuls -> [128, 4]
                # (borrowing an sp-shaped PSUM slot), then one wide reciprocal
                den = pa.tile([128, 512], f32, tag="pa", name="den")
                for s in range(4):
                    nc.tensor.matmul(
                        den[:, s:s + 1],
                        acc[:, s * 128:(s + 1) * 128],
                        onesb[:, :],
                        start=True,
                        stop=True,
                    )
                rinv = ripool.tile([128, 4], f32, tag="ri", name="rinv")
                nc.vector.reciprocal(rinv[:, :], den[:, 0:4])
                osb = obpool.tile([128, 2, 512], f16, tag="osb", name="osb")
                for dc in range(2):
                    nc.vector.tensor_copy(out=osb[:, dc, :], in_=ops[dc][:, :])
                # chunk pairs share one outs tile and flush together
                items = []
                last_blk = (ib == NIB - 1)
                for isub in range(4):
                    for pair in (((0, 512), (512, 512)),
                                 ((1024, 512), (1536, 512)),
                                 ((2048, 256),)):
                        outs = ospool.tile([128, 1024], f16, tag="os",
                                           name="outs")
                        oc = 0
                        for pi, (hc, hw) in enumerate(pair):
                            items.append((osb, rinv, ib, isub, hc, hw, outs,
                                          oc, pi == len(pair) - 1, last_blk))
                            oc += hw
                return items

            pend = []
            prev = None
            for ib in range(NIB):
                qsb = qsbs[ib]
                njc = 4 * ib + 4
                ops = [
                    po.tile([128, 512], f32, tag="po", name="op0"),
                    po.tile([128, 512], f32, tag="po", name="op1"),
                ]
                acc = acpool.tile([128, 512], f32, tag="ac", name="acc")
                pbuf = []

                def av(jc):
                    jb, js = jc // 4, jc % 4
                    first, last = (jc == 0), (jc == njc - 1)
                    for dc in range(2):
                        nc.tensor.matmul(
                            ops[dc][:, :],
                            vts[jb][:, js, dc * 128:(dc + 1) * 128],
                            pbuf[jc][:, :],
                            start=first,
                            stop=last,
                        )

                for jc in range(njc):
                    jb, js = jc // 4, jc % 4
                    sp = pa.tile([128, 512], f32, tag="pa", name="sp")
                    for dc in range(2):
                        nc.tensor.matmul(
                            sp[:, :],
                            kts[jb][:, dc, js * 128:(js + 1) * 128],
                            qsb[:, dc, :],
                            start=(dc == 0),
                            stop=(dc == 1),
                        )
                    th = thpool.tile([128, 512], f32, tag="th", name="th")
                    nc.scalar.activation(th[:, :], sp[:, :], TANH, scale=SCALE / SOFTCAP)
                    p = ppool.tile([128, 512], bf16, tag="pp", name="p")
                    nc.scalar.activation(p[:, :], th[:, :], EXP, scale=SOFTCAP)
                    if jb == ib:  # diagonal block: causal mask via 0/1 multiply
                        pm = ppool.tile([128, 512], bf16, tag="pp", name="pm")
                        nc.vector.tensor_mul(
                            pm[:, :], p[:, :], tri[:, js * 512:(js + 1) * 512]
                        )
                        p = pm
                    pbuf.append(p)
                    # denominator accumulates on the otherwise-idle gpsimd
                    if jc == 0:
                        nc.gpsimd.tensor_copy(out=acc[:, :], in_=p[:, :])
                    else:
                        nc.gpsimd.tensor_add(acc[:, :], acc[:, :], p[:, :])
                    # previous block's denominator at jc==0 (its gpsimd chain
                    # has drained by now), wo projection spread from jc==1;
                    # this block's V transposes slot in at jc==2
                    if jc == 0 and prev is not None:
                        pend = make_den(*prev)
                        prev = None
                    if jc == 2 and ib in vtts:
                        unpack_transpose(ib)
                    if jc >= 1:
                        for _ in range(3):
                            if pend:
                                emit_norm_item(*pend.pop(0))
                    if jc >= 3:
                        av(jc - 3)
                av(njc - 3)
                av(njc - 2)
                av(njc - 1)
                while pend:
                    emit_norm_item(*pend.pop(0))
                prev = (ops, acc, ib)
                if ib + 1 < NIB:
                    unpack_load(ib + 1)
            pend = make_den(*prev)
            while pend:
                emit_norm_item(*pend.pop(0))
    nc.compile()
    return nc


def _host_prep(x, wq, wk, wv, wo):
    """Build per-core input maps (head h on core h; pair 2g/2g+1 shares kv g:
    even core carries wk, odd carries wv)."""
    x2 = x[0, LI:, :]                                   # [2048, 2304]
    x2t = np.ascontiguousarray(x2.T).astype(np.float16)  # [2304, 2048]

    inv_freq = 1.0 / (ROPE_BASE ** (np.arange(0, D, 2, dtype=np.float32) / D))
    t = np.arange(LI, L, dtype=np.float32)
    freqs = np.outer(t, inv_freq)                        # [2048, 128]
    # both 128-row halves of the full [2048, 256] table are identical; the
    # device shares one copy
    cost = np.ascontiguousarray(np.cos(freqs).astype(np.float32).T).astype(np.float16)
    sint = np.ascontiguousarray(np.sin(freqs).astype(np.float32).T).astype(np.float16)

    tri = np.zeros((128, 2048), dtype=_BF16)
    jj = np.arange(128)[:, None]
    ii = np.arange(512)[None, :]
    for k in range(4):
        tri[:, k * 512:(k + 1) * 512] = (128 * k + jj <= ii).astype(_BF16)

    onesb = np.ones((128, 1), dtype=np.float32)
    iden = np.eye(128, dtype=np.float16)
    wrm = np.zeros((128, 16), dtype=np.float16)

    def tile_w(w):  # [2304, 256] -> SBUF layout [128, 18, 256]
        return np.ascontiguousarray(
            w.reshape(HID // 128, 128, D).transpose(1, 0, 2)).astype(np.float16)

    def tile_wo(w):  # [256, 2304] -> SBUF layout [128, 2, 2304]
        return np.ascontiguousarray(
            w.reshape(2, 128, HID).transpose(1, 0, 2)).astype(np.float16)

    in_maps = []
    for h in range(H):
        g = h // 2
        wkv_src = wk if h % 2 == 0 else wv
        in_maps.append({
            "x2t": x2t,
            "wq": tile_w(wq[:, h * D:(h + 1) * D]),
            "wkv": tile_w(wkv_src[:, g * D:(g + 1) * D]),
            "wo": tile_wo(wo[h * D:(h + 1) * D, :]),
            "cost": cost,
            "sint": sint,
            "tri": tri,
            "onesb": onesb,
            "iden": iden,
            "wrm": wrm,
        })
    return in_maps


def _first_half_row(x, wv, wo):
    """Rows 0..2047 of the output: uniform attention over all 4096 keys."""
    vmean = x[0].mean(axis=0, dtype=np.float64).astype(np.float32) @ wv  # [1024]
    per_kv = vmean.reshape(HKV, D)
    o = np.concatenate([per_kv[h // 2] for h in range(H)])  # [2048]
    return o @ wo                                           # [2304]


def _mask_is_causal(mask):
    m = mask[0, 0]
    causal = np.triu(np.full((L, L), np.float32(NEG), dtype=np.float32), k=1)
    return np.array_equal(m, causal)


def _numpy_fallback(x, mask, wq, wk, wv, wo):
    """Direct fp32 replication of the reference (only used if mask is unusual)."""
    xb = x[0]
    q = (xb @ wq).reshape(L, H, D)
    k = (xb @ wk).reshape(L, HKV, D)
    v = (xb @ wv).reshape(L, HKV, D)
    inv_freq = 1.0 / (ROPE_BASE ** (np.arange(0, D, 2, dtype=np.float32) / D))
    t = np.arange(L, dtype=np.float32)
    emb = np.concatenate([np.outer(t, inv_freq)] * 2, axis=-1)
    cos = np.cos(emb).astype(np.float32)[:, None, :]
    sin = np.sin(emb).astype(np.float32)[:, None, :]

    def rope(a):
        a1, a2 = a[..., :D // 2], a[..., D // 2:]
        return a * cos + np.concatenate([-a2, a1], axis=-1) * sin

    q, k = rope(q), rope(k)
    col_keep = np.arange(L) >= (L - 2048)
    out = np.zeros((L, H * D), dtype=np.float32)
    for h in range(H):
        g = h // 2
        s = (q[:, h] @ k[:, g].T) * np.float32(SCALE)
        s = np.float32(SOFTCAP) * np.tanh(s / np.float32(SOFTCAP))
        s = s + mask[0, 0]
        s = np.where(col_keep[None, :], s, np.float32(NEG))
        s = s - s.max(axis=1, keepdims=True)
        p = np.exp(s)
        p /= p.sum(axis=1, keepdims=True)
        out[:, h * D:(h + 1) * D] = p @ v[:, g]
    return (out @ wo).reshape(1, L, HID)


def _run_device(in_maps, trace=False, trace_cores=None):
    from concourse.bass_utils import run_bass_kernel_spmd

    if "nc" not in _CACHE:
        _CACHE["nc"] = _build_nc()
    nc = _CACHE["nc"]
    return run_bass_kernel_spmd(
        nc, in_maps, list(range(H)), trace=trace, trace_cores=trace_cores
    )


def kernel(x, mask, wq, wk, wv, wo):
    x = np.asarray(x, dtype=np.float32)
    mask = np.asarray(mask, dtype=np.float32)
    wq = np.asarray(wq, dtype=np.float32)
    wk = np.asarray(wk, dtype=np.float32)
    wv = np.asarray(wv, dtype=np.float32)
    wo = np.asarray(wo, dtype=np.float32)

    if not _mask_is_causal(mask):
        return _numpy_fallback(x, mask, wq, wk, wv, wo)

    in_maps = _host_prep(x, wq, wk, wv, wo)
    res = _run_device(in_maps)
    parts = np.zeros((LI, HID), dtype=np.float32)
    for c in range(H):
        parts += res.results[c]["part"].astype(np.float32)

    out = np.empty((1, L, HID), dtype=np.float32)
    out[0, :LI, :] = _first_half_row(x, wv, wo)[None, :]
    out[0, LI:, :] = parts
    return out



# revision 23
# speedup vs baseline: 1.0750x; 1.0350x over previous
"""Gemma2 sliding-window attention (B=1, L=4096, H=8/KV4, D=256, HID=2304, W=2048)
on 8 TRN2 NeuronCores via Bass/Tile.

Key structural facts of the reference (validated against it numerically):
- The window mask keeps only key columns >= 2048 for ALL rows; combined with
  the causal mask, rows < 2048 end up with every logit == -1e9 exactly in fp32
  (|softcapped score| < 32 < ulp(1e9)/2), so softmax is uniform over all 4096
  keys: rows 0..2047 of the output are one constant row = colmean(v) @ wo.
- Rows >= 2048 are standard causal softcapped attention over keys [2048, i];
  the -1e9 terms underflow to exactly 0 in the fp32 softmax.
- Softcap bounds logits to [-50, 50], so exp() without max-subtraction is safe
  in fp32 and matches the reference softmax up to rounding.

Sharding: one query head per core. The K/V projections for kv head g=h//2 are
deduplicated across the pair (2g, 2g+1): the even core projects K^T, the odd
core projects V^T (identical SPMD programs; only the per-core weight data
differ), exchanged per 512-block with on-device pair AllGathers (a tiny warmup
collective at kernel start absorbs the ~30us one-time rendezvous). Both cores
then rope the gathered K^T (slot 0) with the shared rope tables and transpose
V^T (slot 1) into V [j, d] layout with PE transposes. kv/q projection blocks
are interleaved so the x2t input stream paces evenly, and warmup matmuls on
scratch data bring the PE to full clock before real work.

Phase 2 computes scores in [j_part, i_free] layout, unnormalized oT in PSUM,
projects through this head's wo slice (the previous block's 20 projection
chunks are spread 3-per-score-chunk through the loop to keep every queue
shallow), and normalizes per query row with a per-partition 1/denominator
scale fused into the PSUM->SBUF drain, alternating vector/scalar. The
denominator accumulates on gpsimd and collapses to a transposed [128, 4]
vector with four K=128 ones-matmuls per i-block. The gathered K/V blocks
unpack lazily inside phase 2 (block b is only needed near the end of phase-2
block b), hiding the collective latency. fp16 partials [2048, 2304] stream
out; host sums the 8 partials in fp32 and prepends the constant first-half
row.
"""
import sys

sys.path.insert(0, "/opt/trn_rl_repo")

import numpy as np
import ml_dtypes

H = 8
HKV = 4
D = 256
HID = 2304
L = 4096
LI = 2048          # second-half rows (local)
NCC = HID // 128   # 18 contraction chunks
NIB = LI // 512    # 4 i-blocks of 512
SCALE = (HID // H) ** -0.5
SOFTCAP = 50.0
NEG = -1e9
ROPE_BASE = 10000.0

_BF16 = ml_dtypes.bfloat16

_CACHE = {}

PAIRS = [[0, 1], [2, 3], [4, 5], [6, 7]]


def _hid_chunks():
    out = []
    c = 0
    while c < HID:
        w = min(512, HID - c)
        out.append((c, w))
        c += w
    return out


def _build_nc():
    import concourse.bass as bass
    import concourse.mybir as mybir
    import concourse.tile as tile
    from concourse import bacc

    f32 = mybir.dt.float32
    f16 = mybir.dt.float16
    bf16 = mybir.dt.bfloat16

    nc = bacc.Bacc("TRN2", target_bir_lowering=False, debug=False, num_devices=8)

    # weights arrive pre-tiled in SBUF layout so each loads with one DMA of
    # 4KB+ per-partition lines (~370 GB/s vs ~15 GB/s for 256B lines)
    x2t_d = nc.dram_tensor("x2t", [HID, LI], f16, kind="ExternalInput").ap()
    wq_d = nc.dram_tensor("wq", [128, NCC, D], f16, kind="ExternalInput").ap()
    wkv_d = nc.dram_tensor("wkv", [128, NCC, D], f16, kind="ExternalInput").ap()
    wo_d = nc.dram_tensor("wo", [128, 2, HID], f16, kind="ExternalInput").ap()
    # rope tables: rows d and d+128 of the [D, LI] table are identical, so
    # only the first 128 rows are stored and shared by both halves
    cos_d = nc.dram_tensor("cost", [128, LI], f16, kind="ExternalInput").ap()
    sin_d = nc.dram_tensor("sint", [128, LI], f16, kind="ExternalInput").ap()
    tri_d = nc.dram_tensor("tri", [128, 2048], bf16, kind="ExternalInput").ap()
    onesb_d = nc.dram_tensor("onesb", [128, 1], f32, kind="ExternalInput").ap()
    iden_d = nc.dram_tensor("iden", [128, 128], f16, kind="ExternalInput").ap()
    wrm_d = nc.dram_tensor("wrm", [128, 16], f16, kind="ExternalInput").ap()
    part_d = nc.dram_tensor("part", [LI, HID], f16, kind="ExternalOutput").ap()

    x2t_r = x2t_d.rearrange("(n p) i -> p n i", p=128)   # [128, 18, 2048]

    TANH = mybir.ActivationFunctionType.Tanh
    EXP = mybir.ActivationFunctionType.Exp
    COPY = mybir.ActivationFunctionType.Copy

    with tile.TileContext(nc) as tc:
        with (
            tc.tile_pool(name="const", bufs=1) as cpool,
            tc.tile_pool(name="kv", bufs=1) as kvpool,
            tc.tile_pool(name="kvs", bufs=2) as kvspool,
            tc.tile_pool(name="qs", bufs=2) as qpool,
            tc.tile_pool(name="th", bufs=5) as thpool,
            tc.tile_pool(name="pp", bufs=6) as ppool,
            tc.tile_pool(name="ob", bufs=2) as obpool,
            tc.tile_pool(name="os", bufs=6) as ospool,
            tc.tile_pool(name="ac", bufs=2) as acpool,
            tc.tile_pool(name="ri", bufs=2) as ripool,
            tc.tile_pool(name="dram", bufs=1, space="DRAM") as dram,
            tc.tile_pool(name="pq", bufs=2, space="PSUM") as pq,
            tc.tile_pool(name="pa", bufs=2, space="PSUM") as pa,
            tc.tile_pool(name="po", bufs=2, space="PSUM") as po,
            tc.tile_pool(name="pt", bufs=2, space="PSUM") as pt,
        ):
            # DRAM bounce buffers for the pair AllGathers (one per i-block)
            kv_bi = dram.tile([NIB, 128, 2, 512], f16)
            kv_bo = dram.tile([NIB, 2, 128, 2, 512], f16)
            wrm_bi = dram.tile([128, 16], f16)
            wrm_bo = dram.tile([2, 128, 16], f16)

            # scratch for PE warmup + act-table warmup (zeros); memset goes
            # FIRST on the gpsimd queue so the PE warmup isn't stuck behind
            # the warmup-collective enqueue
            scratch = cpool.tile([128, 640], f16, tag="scratch")
            nc.gpsimd.memset(scratch[:, :], 0.0)

            # warmup collective: absorbs the one-time CC rendezvous latency
            nc.sync.dma_start(out=wrm_bi[:, :], in_=wrm_d)
            nc.gpsimd.collective_compute(
                "AllGather",
                mybir.AluOpType.bypass,
                replica_groups=PAIRS,
                ins=[wrm_bi[:, :].opt()],
                outs=[wrm_bo[:, :, :].opt()],
            )

            warm = thpool.tile([128, 8], f32, tag="th", name="warm")
            nc.scalar.activation(warm[:, :], scratch[:, 0:8], TANH,
                                 scale=SCALE / SOFTCAP)
            nc.scalar.activation(warm[:, :], warm[:, :], EXP, scale=SOFTCAP)
            # spin the PE up to full clock before the first real matmul
            for w in range(22):
                wp = pq.tile([128, 512], f32, tag="pq", name="wp")
                nc.tensor.matmul(
                    wp[:, :], scratch[:, 0:128], scratch[:, 128:640],
                    start=True, stop=True,
                )

            # ---- resident loads, spread across the three DGE rings (sync,
            # scalar, gpsimd) with fat per-partition lines ----
            x2t = cpool.tile([128, NCC, LI], f16, tag="x2t")
            wkv = cpool.tile([128, NCC, D], f16, tag="wkv")
            wq = cpool.tile([128, NCC, D], f16, tag="wq")
            # scalar ring: pre-tiled weights, one fast DMA each, in order of
            # first use (wkv immediately, wq ~8us, wo at first norm item)
            nc.scalar.dma_start(out=wkv[:, :, :], in_=wkv_d)
            nc.scalar.dma_start(out=wq[:, :, :], in_=wq_d)
            wo = cpool.tile([128, 2, HID], f16, tag="wo")
            nc.scalar.dma_start(out=wo[:, :, :], in_=wo_d)
            # sync ring: i-block 0 of x2t per contraction chunk (feeds the
            # first kv/q projections chunk by chunk)
            for cc in range(NCC):
                nc.sync.dma_start(out=x2t[:, cc, 0:512], in_=x2t_r[:, cc, 0:512])
            # i-blocks 1-3 (3KB lines) round-robin across the three rings so
            # no single ring serializes the 7MB; rope tables slot in early on
            # gpsimd (needed for q0 rope ~15us in)
            cos = cpool.tile([128, LI], f16, tag="cos")
            sin = cpool.tile([128, LI], f16, tag="sin")
            rings = (nc.gpsimd, nc.scalar, nc.sync)
            for cc in range(NCC):
                rings[cc % 3].dma_start(
                    out=x2t[:, cc, 512:2048], in_=x2t_r[:, cc, 512:2048]
                )
                if cc == 4:
                    nc.gpsimd.dma_start(out=cos[:, :], in_=cos_d)
                    nc.gpsimd.dma_start(out=sin[:, :], in_=sin_d)
            # sync ring continues with late-needed consts
            iden = cpool.tile([128, 128], f16, tag="iden")
            nc.sync.dma_start(out=iden[:, :], in_=iden_d)
            tri = cpool.tile([128, 2048], bf16, tag="tri")
            nc.sync.dma_start(out=tri[:, :], in_=tri_d)
            onesb = cpool.tile([128, 1], f32, tag="onesb")
            nc.sync.dma_start(out=onesb[:, :], in_=onesb_d)

            # per-i-block persistent K^T (fp16, [d_chunk, j]) and V (bf16, [j, d])
            kts = [
                kvpool.tile([128, 2, 512], f16, tag=f"kt{b}", name=f"kt{b}")
                for b in range(NIB)
            ]
            vts = [
                kvpool.tile([128, 4, D], bf16, tag=f"vt{b}", name=f"vt{b}")
                for b in range(NIB)
            ]

            qsbs = [
                qpool.tile([128, 2, 512], f16, tag=f"qsb{b}", name=f"qsb{b}")
                for b in range(NIB)
            ]

            def rope_sb(c0, c1, out0, out1, isl):
                # out0 = c0*cos - c1*sin ; out1 = c1*cos + c0*sin
                # (cos/sin identical for both 128-row halves of the head dim)
                for dst, a, b_, op in ((0, c0, c1, "sub"), (1, c1, c0, "add")):
                    ta = thpool.tile([128, 512], f32, tag="th", name="ta")
                    nc.vector.tensor_mul(ta[:, :], a, cos[:, isl])
                    tb = thpool.tile([128, 512], f32, tag="th", name="tb")
                    nc.vector.tensor_mul(tb[:, :], b_, sin[:, isl])
                    dstap = out0 if dst == 0 else out1
                    if op == "sub":
                        nc.vector.tensor_sub(dstap, ta[:, :], tb[:, :])
                    else:
                        nc.vector.tensor_add(dstap, ta[:, :], tb[:, :])

            def proj(wt, dst_sb):
                # dst_sb[:, dc, :] = (wt.T @ x2t_block) via scalar PSUM drain
                for dc in range(2):
                    pp_ = pq.tile([128, 512], f32, tag="pq", name="prj")
                    for cc in range(NCC):
                        nc.tensor.matmul(
                            pp_[:, :],
                            wt[:, cc, dc * 128:(dc + 1) * 128],
                            x2t[:, cc, dst_sb[1]],
                            start=(cc == 0),
                            stop=(cc == NCC - 1),
                        )
                    nc.scalar.activation(dst_sb[0][:, dc, :], pp_[:, :], COPY)

            # ===== phase 1: kv blocks front-loaded so the pair AllGathers
            # chain right behind the warmup rendezvous; q blocks fill in =====
            # kv(b): K^T on even cores / V^T on odd (pre-rope), -> AllGather b
            # q(b):  this head's Q block + rope
            def kv_block(b):
                isl = slice(b * 512, (b + 1) * 512)
                kvt = kvspool.tile([128, 2, 512], f16, tag="kvt", name="kvt")
                proj(wkv, (kvt, isl))
                # bounce write on the scalar DGE ring: the data was just
                # produced by scalar copies, and the sync ring is backlogged
                # with the resident-load stream
                nc.scalar.dma_start(out=kv_bi[b, :, :, :], in_=kvt[:, :, :])
                nc.gpsimd.collective_compute(
                    "AllGather",
                    mybir.AluOpType.bypass,
                    replica_groups=PAIRS,
                    ins=[kv_bi[b, :, :, :].opt()],
                    outs=[kv_bo[b, :, :, :, :].opt()],
                )

            def q_block(b):
                isl = slice(b * 512, (b + 1) * 512)
                qc = kvspool.tile([128, 2, 512], f16, tag="qc", name="qc")
                proj(wq, (qc, isl))
                qsb = qsbs[b]
                rope_sb(qc[:, 0, :], qc[:, 1, :], qsb[:, 0, :], qsb[:, 1, :], isl)

            # kv blocks front-loaded: every pair AllGather is enqueued as
            # early as possible so the serial CC chain finishes before the
            # attention blocks need the gathered K/V
            kv_block(0)
            kv_block(1)
            q_block(0)
            kv_block(2)
            q_block(1)
            kv_block(3)
            q_block(2)
            q_block(3)

            # lazy unpack: block b's K^T rope / V^T transposes are only needed
            # near the END of phase-2 block b, so they interleave into phase 2
            flip = [0]
            vtts = {}

            def unpack_load(b):
                # DRAM->SBUF loads + K rope (vector); gated on collective b
                krsb = kvspool.tile([128, 2, 512], f16, tag="kr", name=f"kr{b}")
                nc.sync.dma_start(out=krsb[:, :, :], in_=kv_bo[b, 0, :, :, :])
                psl = slice(b * 512, (b + 1) * 512)
                rope_sb(krsb[:, 0, :], krsb[:, 1, :],
                        kts[b][:, 0, :], kts[b][:, 1, :], psl)
                vtt = kvspool.tile([128, 2, 512], f16, tag="vtt", name=f"vtt{b}")
                nc.sync.dma_start(out=vtt[:, :, :], in_=kv_bo[b, 1, :, :, :])
                vtts[b] = vtt

            def unpack_transpose(jb):
                vtt = vtts.pop(jb)
                for js in range(4):
                    tp = pt.tile([128, 256], f16, tag="pt", name="tp")
                    for dc in range(2):
                        nc.tensor.transpose(
                            tp[:, dc * 128:(dc + 1) * 128],
                            vtt[:, dc, js * 128:(js + 1) * 128],
                            iden[:, :],
                        )
                    dst = vts[jb][:, js, :]
                    if flip[0] % 2 == 0:
                        nc.scalar.activation(dst, tp[:, :], COPY)
                    else:
                        nc.vector.tensor_copy(out=dst, in_=tp[:, :])
                    flip[0] += 1

            unpack_load(0)
            unpack_transpose(0)

            # ===== phase 2: attention + output projection, software-pipelined =====
            mflip = [0]
            dflip = [0]
            dflush = []

            def emit_norm_item(osb, rinv, ib, isub, hc, hw, outs, oc, flush,
                               last_blk):
                outp = pq.tile([128, hw], f32, tag="pq", name="outp")
                for dc in range(2):
                    nc.tensor.matmul(
                        outp[:, :],
                        osb[:, dc, isub * 128:(isub + 1) * 128],
                        wo[:, dc, hc:hc + hw],
                        start=(dc == 0),
                        stop=(dc == 1),
                    )
                # PSUM drain + 1/den scale, alternating vector/scalar so
                # neither engine paces the outp ring
                if mflip[0] % 2 == 0:
                    nc.vector.tensor_scalar_mul(
                        outs[:, oc:oc + hw], outp[:, :], rinv[:, isub:isub + 1]
                    )
                else:
                    nc.scalar.mul(outs[:, oc:oc + hw], outp[:, :],
                                  rinv[:, isub:isub + 1])
                mflip[0] += 1
                if flush:
                    # queue the pair write (2KB per-partition lines); actual
                    # emission is deferred a couple of items so the DMA's
                    # drain-semaphore wait is ~0 at the queue head. The final
                    # i-block alternates rings so its tail drains 2x.
                    if last_blk:
                        eng = nc.sync if dflip[0] % 2 == 0 else nc.scalar
                        dflip[0] += 1
                    else:
                        eng = nc.sync
                    w = oc + hw              # total cols in this outs tile
                    base = hc - oc           # output col of outs[:, 0]
                    r0 = ib * 512 + isub * 128
                    dflush.append((eng, r0, base, w, outs))

            def emit_flush(spec):
                eng, r0, base, w, outs = spec
                eng.dma_start(
                    out=part_d[r0:r0 + 128, base:base + w],
                    in_=outs[:, :w],
                )

            def make_den(ops, acc, ib):
                # transposed denominator: 4 K=128 ones-matmuls -> [128, 4]
                # (borrowing an sp-shaped PSUM slot), then one wide reciprocal
                den = pa.tile([128, 512], f32, tag="pa", name="den")
                for s in range(4):
                    nc.tensor.matmul(
                        den[:, s:s + 1],
                        acc[:, s * 128:(s + 1) * 128],
                        onesb[:, :],
                        start=True,
                        stop=True,
                    )
                rinv = ripool.tile([128, 4], f32, tag="ri", name="rinv")
                nc.vector.reciprocal(rinv[:, :], den[:, 0:4])
                osb = obpool.tile([128, 2, 512], f16, tag="osb", name="osb")
                for dc in range(2):
                    nc.vector.tensor_copy(out=osb[:, dc, :], in_=ops[dc][:, :])
                # chunk pairs share one outs tile and flush together
                items = []
                last_blk = (ib == NIB - 1)
                for isub in range(4):
                    for pair in (((0, 512), (512, 512)),
                                 ((1024, 512), (1536, 512)),
                                 ((2048, 256),)):
                        outs = ospool.tile([128, 1024], f16, tag="os",
                                           name="outs")
                        oc = 0
                        for pi, (hc, hw) in enumerate(pair):
                            items.append((osb, rinv, ib, isub, hc, hw, outs,
                                          oc, pi == len(pair) - 1, last_blk))
                            oc += hw
                return items

            pend = []
            prev = None
            for ib in range(NIB):
                qsb = qsbs[ib]
                njc = 4 * ib + 4
                ops = [
                    po.tile([128, 512], f32, tag="po", name="op0"),
                    po.tile([128, 512], f32, tag="po", name="op1"),
                ]
                acc = acpool.tile([128, 512], f32, tag="ac", name="acc")
                pbuf = []

                def av(jc):
                    jb, js = jc // 4, jc % 4
                    first, last = (jc == 0), (jc == njc - 1)
                    for dc in range(2):
                        nc.tensor.matmul(
                            ops[dc][:, :],
                            vts[jb][:, js, dc * 128:(dc + 1) * 128],
                            pbuf[jc][:, :],
                            start=first,
                            stop=last,
                        )

                for jc in range(njc):
                    jb, js = jc // 4, jc % 4
                    sp = pa.tile([128, 512], f32, tag="pa", name="sp")
                    for dc in range(2):
                        nc.tensor.matmul(
                            sp[:, :],
                            kts[jb][:, dc, js * 128:(js + 1) * 128],
                            qsb[:, dc, :],
                            start=(dc == 0),
                            stop=(dc == 1),
                        )
                    th = thpool.tile([128, 512], f32, tag="th", name="th")
                    nc.scalar.activation(th[:, :], sp[:, :], TANH, scale=SCALE / SOFTCAP)
                    p = ppool.tile([128, 512], bf16, tag="pp", name="p")
                    nc.scalar.activation(p[:, :], th[:, :], EXP, scale=SOFTCAP)
                    if jb == ib:  # diagonal block: causal mask via 0/1 multiply
                        pm = ppool.tile([128, 512], bf16, tag="pp", name="pm")
                        nc.vector.tensor_mul(
                            pm[:, :], p[:, :], tri[:, js * 512:(js + 1) * 512]
                        )
                        p = pm
                    pbuf.append(p)
                    # denominator accumulates on the otherwise-idle gpsimd
                    if jc == 0:
                        nc.gpsimd.tensor_copy(out=acc[:, :], in_=p[:, :])
                    else:
                        nc.gpsimd.tensor_add(acc[:, :], acc[:, :], p[:, :])
                    # previous block's denominator at jc==0 (its gpsimd chain
                    # has drained by now), wo projection spread from jc==1;
                    # this block's V transposes slot in at jc==2
                    if jc == 0 and prev is not None:
                        pend = make_den(*prev)
                        prev = None
                    if jc == 2 and ib in vtts:
                        unpack_transpose(ib)
                    if jc >= 1:
                        for _ in range(3):
                            if pend:
                                emit_norm_item(*pend.pop(0))
                        while len(dflush) > 2:
                            emit_flush(dflush.pop(0))
                    if jc >= 3:
                        av(jc - 3)
                av(njc - 3)
                av(njc - 2)
                av(njc - 1)
                while pend:
                    emit_norm_item(*pend.pop(0))
                while len(dflush) > 2:
                    emit_flush(dflush.pop(0))
                prev = (ops, acc, ib)
                if ib + 1 < NIB:
                    unpack_load(ib + 1)
            pend = make_den(*prev)
            while pend:
                emit_norm_item(*pend.pop(0))
            while dflush:
                emit_flush(dflush.pop(0))
    nc.compile()
    return nc


def _host_prep(x, wq, wk, wv, wo):
    """Build per-core input maps (head h on core h; pair 2g/2g+1 shares kv g:
    even core carries wk, odd carries wv)."""
    x2 = x[0, LI:, :]                                   # [2048, 2304]
    x2t = np.ascontiguousarray(x2.T).astype(np.float16)  # [2304, 2048]

    inv_freq = 1.0 / (ROPE_BASE ** (np.arange(0, D, 2, dtype=np.float32) / D))
    t = np.arange(LI, L, dtype=np.float32)
    freqs = np.outer(t, inv_freq)                        # [2048, 128]
    # both 128-row halves of the full [2048, 256] table are identical; the
    # device shares one copy
    cost = np.ascontiguousarray(np.cos(freqs).astype(np.float32).T).astype(np.float16)
    sint = np.ascontiguousarray(np.sin(freqs).astype(np.float32).T).astype(np.float16)

    tri = np.zeros((128, 2048), dtype=_BF16)
    jj = np.arange(128)[:, None]
    ii = np.arange(512)[None, :]
    for k in range(4):
        tri[:, k * 512:(k + 1) * 512] = (128 * k + jj <= ii).astype(_BF16)

    onesb = np.ones((128, 1), dtype=np.float32)
    iden = np.eye(128, dtype=np.float16)
    wrm = np.zeros((128, 16), dtype=np.float16)

    def tile_w(w):  # [2304, 256] -> SBUF layout [128, 18, 256]
        return np.ascontiguousarray(
            w.reshape(HID // 128, 128, D).transpose(1, 0, 2)).astype(np.float16)

    def tile_wo(w):  # [256, 2304] -> SBUF layout [128, 2, 2304]
        return np.ascontiguousarray(
            w.reshape(2, 128, HID).transpose(1, 0, 2)).astype(np.float16)

    in_maps = []
    for h in range(H):
        g = h // 2
        wkv_src = wk if h % 2 == 0 else wv
        in_maps.append({
            "x2t": x2t,
            "wq": tile_w(wq[:, h * D:(h + 1) * D]),
            "wkv": tile_w(wkv_src[:, g * D:(g + 1) * D]),
            "wo": tile_wo(wo[h * D:(h + 1) * D, :]),
            "cost": cost,
            "sint": sint,
            "tri": tri,
            "onesb": onesb,
            "iden": iden,
            "wrm": wrm,
        })
    return in_maps


def _first_half_row(x, wv, wo):
    """Rows 0..2047 of the output: uniform attention over all 4096 keys."""
    vmean = x[0].mean(axis=0, dtype=np.float64).astype(np.float32) @ wv  # [1024]
    per_kv = vmean.reshape(HKV, D)
    o = np.concatenate([per_kv[h // 2] for h in range(H)])  # [2048]
    return o @ wo                                           # [2304]


def _mask_is_causal(mask):
    m = mask[0, 0]
    causal = np.triu(np.full((L, L), np.float32(NEG), dtype=np.float32), k=1)
    return np.array_equal(m, causal)


def _numpy_fallback(x, mask, wq, wk, wv, wo):
    """Direct fp32 replication of the reference (only used if mask is unusual)."""
    xb = x[0]
    q = (xb @ wq).reshape(L, H, D)
    k = (xb @ wk).reshape(L, HKV, D)
    v = (xb @ wv).reshape(L, HKV, D)
    inv_freq = 1.0 / (ROPE_BASE ** (np.arange(0, D, 2, dtype=np.float32) / D))
    t = np.arange(L, dtype=np.float32)
    emb = np.concatenate([np.outer(t, inv_freq)] * 2, axis=-1)
    cos = np.cos(emb).astype(np.float32)[:, None, :]
    sin = np.sin(emb).astype(np.float32)[:, None, :]

    def rope(a):
        a1, a2 = a[..., :D // 2], a[..., D // 2:]
        return a * cos + np.concatenate([-a2, a1], axis=-1) * sin

    q, k = rope(q), rope(k)
    col_keep = np.arange(L) >= (L - 2048)
    out = np.zeros((L, H * D), dtype=np.float32)
    for h in range(H):
        g = h // 2
        s = (q[:, h] @ k[:, g].T) * np.float32(SCALE)
        s = np.float32(SOFTCAP) * np.tanh(s / np.float32(SOFTCAP))
        s = s + mask[0, 0]
        s = np.where(col_keep[None, :], s, np.float32(NEG))
        s = s - s.max(axis=1, keepdims=True)
        p = np.exp(s)
        p /= p.sum(axis=1, keepdims=True)
        out[:, h * D:(h + 1) * D] = p @ v[:, g]
    return (out @ wo).reshape(1, L, HID)


def _run_device(in_maps, trace=False, trace_cores=None):
    from concourse.bass_utils import run_bass_kernel_spmd

    if "nc" not in _CACHE:
        _CACHE["nc"] = _build_nc()
    nc = _CACHE["nc"]
    return run_bass_kernel_spmd(
        nc, in_maps, list(range(H)), trace=trace, trace_cores=trace_cores
    )


def kernel(x, mask, wq, wk, wv, wo):
    x = np.asarray(x, dtype=np.float32)
    mask = np.asarray(mask, dtype=np.float32)
    wq = np.asarray(wq, dtype=np.float32)
    wk = np.asarray(wk, dtype=np.float32)
    wv = np.asarray(wv, dtype=np.float32)
    wo = np.asarray(wo, dtype=np.float32)

    if not _mask_is_causal(mask):
        return _numpy_fallback(x, mask, wq, wk, wv, wo)

    in_maps = _host_prep(x, wq, wk, wv, wo)
    res = _run_device(in_maps)
    parts = np.zeros((LI, HID), dtype=np.float32)
    for c in range(H):
        parts += res.results[c]["part"].astype(np.float32)

    out = np.empty((1, L, HID), dtype=np.float32)
    out[0, :LI, :] = _first_half_row(x, wv, wo)[None, :]
    out[0, LI:, :] = parts
    return out

